# revision 3
# baseline (speedup 1.0000x reference)
"""Trainium2 Bass kernel for DeformationTrackerBiFlowModel — G=7, h-ship.

Reference math (per batch element b, per step t):
    x_t   = [prev_out (2), fin_t (3)]            (5,)
    h_t   = tanh(x_t @ W_rnn + b_rnn)            (12,)   (U_rnn is inert)
    out_t = [cp0 (2), h_t (12)] @ W_out + b_out  (2,)
    prev_out_{t+1} = out_t;  prev_out_0 = cp0

Folded recurrence (h carries the state; out is an affine readout):
    pre_t = h_{t-1} @ Wh + fin_t @ W1f + 1*r + cp0 @ E     Wh = Wo2 @ W1p
    h_t   = tanh(pre_t)
    out_t = cvec + h_t @ Wo2          <- computed on the HOST from shipped h

Device does ONLY the serial part: one K=120 x M=84 matmul + one tanh per
chain-step (no out columns, no DVE, no second matmul).  h_t is written by the
ACT directly into the next step's rhs block; every 8 steps a chain's filled
phase-tile (8 blocks of h) is DMA'd to HBM as one contiguous [84, 8*COLS]
transfer (84 descriptors).  The host computes out = cvec + h @ Wo2 in f32 —
numerically identical to the on-device readout path of the old kernel.

Slot s holds (h_s, fin_{s+1}); MM_t reads slot t-1, ACT_t writes slot t.
Slots live in 4 rotating phase-tiles x 8 blocks per chain (slot -1 = tile 3
block 7, pre-staged with fin_0/zero-h for the w0 step-0 matmul).  fin is
staged in DRAM grouped 4-slots-contiguous so each 4-step prefetch DMA is 21
descriptors of 4*COLS*2B.

Steady state per step: PE 3x(391+62)/1.2 ~ 1.14us, ACT 3x(391+190)/1.2 ~
1.45us (bottleneck), chain latency ~1.05us.  Batch 65536 over 8 cores;
per core G*C*COLS = 7*3*391 = 8211 (8192 + pad 19).
"""

import os
from contextlib import ExitStack

import numpy as np

import concourse.mybir as mybir
import concourse.tile as tile
from concourse import bacc
from concourse.bass_utils import run_bass_kernel_spmd

B, T = 65536, 100
D_CP, D_FIN, HID = 2, 3, 12
NCORES = 8
BC = B // NCORES              # 8192 per core
G = 7                         # trajectories packed per column (block-diag)
C = 3                         # independent column chains
COLS = 391                    # batch columns per chain
BP = G * C * COLS             # 8211 padded batch per core
NH = HID * G                  # 84: h rows (rhs) / pre rows (psum)
NFIN = D_FIN * G              # 21 fin rows
NST = 1 + D_CP * G            # 15 static rows: ones | cp0
KTOT = NH + NFIN + NST        # 120
NPH = 4                       # h phase-tiles per chain
NGRP = (T + 7) // 8           # 13 h-ship groups (last one half)
NFG = T // 4                  # 25 fin groups (slots 4g..4g+3 = steps 4g+1..4g+4)

F32 = mybir.dt.float32

_MM_CHOICES = {"bf16": mybir.dt.bfloat16, "f32r": mybir.dt.float32r, "f32": F32}
MM_DTYPE = _MM_CHOICES[os.environ.get("DTB_MM", "bf16")]
MM_NP = mybir.dt.np(MM_DTYPE)

LAST_RESULTS = None  # test.py introspects profiling info from here


def build_program(t_steps=T, g=G, c=C, cols=COLS, mm_dtype=None):
    if mm_dtype is None:
        mm_dtype = MM_DTYPE
    XDT = mm_dtype
    nh, nfin, nst = HID * g, D_FIN * g, 1 + D_CP * g
    ktot = nh + nfin + nst
    ngrp = (t_steps + 7) // 8
    nfg = t_steps // 4
    nc = bacc.Bacc(target_bir_lowering=False)

    fin = nc.dram_tensor("fin", [c, nfin, nfg, 4 * cols], XDT, kind="ExternalInput")
    fin0 = nc.dram_tensor("fin0", [c, nfin, cols], XDT, kind="ExternalInput")
    xc = nc.dram_tensor("xc", [c, nst, 8 * cols], XDT, kind="ExternalInput")
    w = nc.dram_tensor("w", [ktot, nh], XDT, kind="ExternalInput")
    w0 = nc.dram_tensor("w0", [ktot, nh], XDT, kind="ExternalInput")
    hd = nc.dram_tensor("hd", [ngrp, c, nh, 8 * cols], XDT, kind="ExternalOutput")

    tanh = mybir.ActivationFunctionType.Tanh

    def s_tile(s):  # phase tile of slot s (python floor-div handles s=-1)
        return (s // 8) % NPH

    def s_blk(s):
        return s % 8

    with tile.TileContext(nc) as tc, ExitStack() as ctx:
        const = ctx.enter_context(tc.tile_pool(name="const", bufs=1))
        xpool = ctx.enter_context(tc.tile_pool(name="xpool", bufs=1))
        psum = ctx.enter_context(tc.tile_pool(name="psum", bufs=2, space="PSUM"))

        ws = const.tile([ktot, nh], XDT, name="ws")
        nc.sync.dma_start(out=ws, in_=w[:, :])
        w0s = const.tile([ktot, nh], XDT, name="w0s")
        nc.gpsimd.dma_start(out=w0s, in_=w0[:, :])

        # 4 phase-tiles per chain: rows h 0:84 | fin 84:105 | ones+cp0 105:120,
        # 8 column-blocks each.  Static rows replicated into every phase tile.
        xts = []
        for ch in range(c):
            tiles = []
            for p in range(NPH):
                xt = xpool.tile([ktot, 8 * cols], XDT, tag=f"x{ch}_{p}",
                                name=f"x_{ch}_{p}")
                eng = nc.sync if (ch + p) % 2 == 0 else nc.gpsimd
                eng.dma_start(out=xt[nh + nfin :, :], in_=xc[ch])
                tiles.append(xt)
            xts.append(tiles)
            # slot -1 (= tile 3, block 7): fin_0 + zeroed h rows for step 0.
            x3 = tiles[3]
            nc.vector.memset(x3[0:nh, 7 * cols : 8 * cols], 0)
            nc.sync.dma_start(
                out=x3[nh : nh + nfin, 7 * cols : 8 * cols], in_=fin0[ch]
            )
            # fin groups 0 and 1 (slots 0..7 -> tile 0)
            for gg in range(2):
                nc.sync.dma_start(
                    out=tiles[0][nh : nh + nfin, 4 * gg * cols : (4 * gg + 4) * cols],
                    in_=fin[ch, :, gg, :],
                )

        for t in range(t_steps):
            for ch in range(c):
                xt_r = xts[ch][s_tile(t - 1)]
                rb = s_blk(t - 1)
                p1 = psum.tile([nh, cols], F32, tag=f"p{ch}", name=f"p_{ch}_{t}")
                nc.tensor.matmul(
                    p1, w0s if t == 0 else ws,
                    xt_r[:, rb * cols : (rb + 1) * cols], start=True, stop=True,
                )
                xt_w = xts[ch][s_tile(t)]
                wb = s_blk(t)
                nc.scalar.activation(
                    xt_w[0:nh, wb * cols : (wb + 1) * cols], p1[:, :], tanh
                )
                # Ship a filled phase-tile (8 slots of h) per 8 steps; the
                # tile is reused 32 slots later so the DMA has ~30us.
                if t % 8 == 7:
                    k = t // 8
                    src = xts[ch][k % NPH]
                    nc.sync.dma_start(
                        out=hd[k, ch, 0 : nh // 2, :], in_=src[0 : nh // 2, :]
                    )
                    nc.gpsimd.dma_start(
                        out=hd[k, ch, nh // 2 : nh, :], in_=src[nh // 2 : nh, :]
                    )
                if t == t_steps - 1 and t % 8 != 7:
                    # trailing partial group (slots 8k..t -> tile k%4 blocks)
                    k = t // 8
                    nb = (t % 8) + 1
                    src = xts[ch][k % NPH]
                    nc.sync.dma_start(
                        out=hd[k, ch, 0 : nh // 2, 0 : nb * cols],
                        in_=src[0 : nh // 2, 0 : nb * cols],
                    )
                    nc.gpsimd.dma_start(
                        out=hd[k, ch, nh // 2 : nh, 0 : nb * cols],
                        in_=src[nh // 2 : nh, 0 : nb * cols],
                    )
                # Prefetch fin group g (slots 4g..4g+3), ~6 steps ahead.
                if t % 4 == 3:
                    gg = (t + 5) // 4
                    if gg < nfg:
                        b0 = (4 * gg) % 8
                        nc.sync.dma_start(
                            out=xts[ch][s_tile(4 * gg)][
                                nh : nh + nfin, b0 * cols : (b0 + 4) * cols
                            ],
                            in_=fin[ch, :, gg, :],
                        )
    nc.compile()
    return nc


def build_packed_weights(W_rnn, W_out, b_rnn, b_out, g=G):
    W_rnn = np.asarray(W_rnn, np.float32)
    W_out = np.asarray(W_out, np.float32)
    b_rnn = np.asarray(b_rnn, np.float32)
    b_out = np.asarray(b_out, np.float32)
    W1p, W1f = W_rnn[:D_CP], W_rnn[D_CP:]
    Wo1, Wo2 = W_out[:D_CP], W_out[D_CP:]
    nh, nfin = HID * g, D_FIN * g
    ktot = nh + nfin + 1 + D_CP * g
    ones_row = nh + nfin
    cp0_base = ones_row + 1

    E = Wo1 @ W1p                      # (2, 12) cp0 contribution to pre
    r = b_rnn + b_out @ W1p            # (12,) ones-row weight (steady state)
    Wh = Wo2 @ W1p                     # (12, 12) h contribution to next pre

    w = np.zeros((ktot, nh), np.float32)
    w0 = np.zeros((ktot, nh), np.float32)
    for i in range(g):
        hsl = slice(HID * i, HID * (i + 1))
        w[hsl, hsl] = Wh
        fsl = slice(nh + D_FIN * i, nh + D_FIN * (i + 1))
        w[fsl, hsl] = W1f
        w0[fsl, hsl] = W1f
        w[ones_row, hsl] = r
        w0[ones_row, hsl] = b_rnn
        csl = slice(cp0_base + D_CP * i, cp0_base + D_CP * (i + 1))
        w[csl, hsl] = E
        w0[csl, hsl] = W1p
    return w, w0


def stage_inputs(cp0, fin, g=G, c=C, cols=COLS, t_steps=T):
    """Batch-major -> feature-major device layouts (b = ch*(g*cols)+gi*cols+j)."""
    bp = g * c * cols
    bc = cp0.shape[0]
    nfg = t_steps // 4
    F = np.zeros((bp, t_steps + 1, D_FIN), np.float32)
    F[:bc, :t_steps] = fin
    cp0_p = np.zeros((bp, D_CP), np.float32)
    cp0_p[:bc] = cp0
    # groups g: slots 4g..4g+3 = fin steps 4g+1..4g+4 (step t_steps = zeros)
    fin_d = np.ascontiguousarray(
        F[:, 1:].reshape(c, g, cols, nfg, 4, D_FIN).transpose(0, 1, 5, 3, 4, 2)
    ).reshape(c, D_FIN * g, nfg, 4 * cols)
    fin0_d = np.ascontiguousarray(
        F[:, 0].reshape(c, g, cols, D_FIN).transpose(0, 1, 3, 2)
    ).reshape(c, D_FIN * g, cols)
    xc_d = np.ones((c, 1 + D_CP * g, cols), np.float32)
    xc_d[:, 1:, :] = (
        cp0_p.reshape(c, g, cols, D_CP).transpose(0, 1, 3, 2).reshape(c, D_CP * g, cols)
    )
    xc_d = np.tile(xc_d, (1, 1, 8))
    return fin_d, fin0_d, xc_d


def unstage_output(hd, cvec_p, Wo2, bc, g=G, c=C, cols=COLS, t_steps=T):
    """hd [ngrp, c, 84, 8*cols] bf16 h-slots -> out [bc, T, 2] f32."""
    bp = g * c * cols
    ngrp = hd.shape[0]
    H = hd.reshape(ngrp, c, g, HID, 8, cols).transpose(1, 2, 5, 0, 4, 3)
    H = np.ascontiguousarray(H).reshape(bp, ngrp * 8, HID)[:, :t_steps]
    out = H.astype(np.float32) @ Wo2  # (bp, T, 2)
    out += cvec_p[:, None, :]
    return out[:bc]


def kernel(control_point_input, finger_input, W_rnn, U_rnn, b_rnn, W_out, b_out):
    global LAST_RESULTS
    cp = np.asarray(control_point_input, np.float32)
    fin = np.asarray(finger_input, np.float32)
    W_rnn = np.asarray(W_rnn, np.float32)
    b_rnn = np.asarray(b_rnn, np.float32)
    W_out = np.asarray(W_out, np.float32)
    b_out = np.asarray(b_out, np.float32)

    cp0 = cp[:, 0, :]
    cvec = cp0 @ W_out[:D_CP] + b_out
    Wo2 = W_out[D_CP:]
    w, w0 = build_packed_weights(W_rnn, W_out, b_rnn, b_out)
    w, w0 = (x.astype(MM_NP) for x in (w, w0))

    nc = build_program()
    in_maps = []
    for m in range(NCORES):
        sl = slice(m * BC, (m + 1) * BC)
        fin_d, fin0_d, xc_d = stage_inputs(cp0[sl], fin[sl])
        in_maps.append(
            {"fin": fin_d.astype(MM_NP, copy=False),
             "fin0": fin0_d.astype(MM_NP, copy=False),
             "xc": xc_d.astype(MM_NP, copy=False),
             "w": w, "w0": w0}
        )

    trace = bool(os.environ.get("DTB_TRACE"))
    res = run_bass_kernel_spmd(
        nc, in_maps, core_ids=list(range(NCORES)), trace=trace
    )
    LAST_RESULTS = res

    outs = []
    for m in range(NCORES):
        sl = slice(m * BC, (m + 1) * BC)
        cvec_p = np.zeros((BP, D_CP), np.float32)
        cvec_p[:BC] = cvec[sl]
        outs.append(
            unstage_output(np.asarray(res.results[m]["hd"]), cvec_p, Wo2, BC)
        )
    return np.concatenate(outs, axis=0)
